# revision 8
# baseline (speedup 1.0000x reference)
"""GCN layer (normalized adjacency aggregation) on 8 Trainium2 NeuronCores.

Algorithm (row-sharded, single bf16 residency + fp8 degree scout):
    a_hat = A + I  (identity folded into the shard on the host)
    deg[i] = sum_j a_hat[i, j]     -> fp8 scout copy, DoubleRow PE matmuls
    dinv = deg ** -0.5             -> sqrt + fast-reciprocal, AllGather (4KB)
    sup = x @ W.T + b              -> computed redundantly per core (tiny)
    S = dinv[:, None] * sup
    out[i, :] = dinv[i] * (a_hat[i, :] @ S)

Two copies of the row block are shipped, both transposed (contraction dim j on
partitions) and tile-major with 8KB-contiguous per-partition rows (big DMA
descriptors amortize the per-descriptor overhead):
  * a8  fp8 e4m3 in 8 oct tiles [128, 8, 1024]; 64 DoubleRow ones-matmuls
    produce the degree row sums.  fp8 degree error is ~2e-4 relative.
  * ab  bf16 in 16 quad tiles [128, 4, 1024], streamed second and kept fully
    SBUF-resident (16 MB); the aggregation pass has zero HBM re-reads.
The scout exists so the degree scan (and the AllGather, whose fixed cost is
~13us issue + ~28us execute) overlaps the bf16 stream instead of sitting
exposed between the two PE passes.

Queue plan (only three DMA queues exist: SP-HW, Act-HW, Pool-SW):
  * big streams alternate between the two HW DGE queues; a descriptor that
    waits on a semaphore head-of-line blocks its whole queue, so the
    dinv/AllGather staging lives on the Pool (gpsimd) software queue.
  * xt is emitted mid-ab-stream so the support matmuls become ready during
    the AllGather window instead of competing with the scout scan.
"""

import numpy as np
from contextlib import ExitStack

N = 8192
F = 128
NCORES = 8
RPC = N // NCORES  # 1024 rows per core
P = 128            # partitions
JT = N // P        # 64 j-tiles of 128 columns
OCTS = JT // 8     # 8 fp8 scout tiles of 1024 columns
QUADS = JT // 4    # 16 bf16 tiles of 512 columns

A8_BUFS = 3        # streaming pool for the fp8 scout tiles (8KB/part each)
XT_BUFS = 6
AB_SPLIT = 10      # ab tiles emitted before xt (xt lands mid-stream)


def build_module():
    """Build and compile the SPMD Bass module (same program on every core)."""
    from concourse import bass, bacc, tile

    mybir = bass.mybir
    f32 = mybir.dt.float32
    bf16 = mybir.dt.bfloat16
    f8 = mybir.dt.float8e4

    nc = bacc.Bacc(
        "TRN2",
        target_bir_lowering=False,
        debug=False,
        enable_asserts=False,
        num_devices=NCORES,
    )

    a8_d = nc.dram_tensor("a8", [OCTS * P, 8 * RPC], f8, kind="ExternalInput")
    ab_d = nc.dram_tensor("ab", [QUADS * P, 4 * RPC], bf16, kind="ExternalInput")
    xt_d = nc.dram_tensor("xt", [F, N], bf16, kind="ExternalInput")
    wt_d = nc.dram_tensor("wt", [F, F], bf16, kind="ExternalInput")
    b4_d = nc.dram_tensor("bias4", [1, 4 * F], f32, kind="ExternalInput")
    ones_r_d = nc.dram_tensor("ones_r", [1, P], f32, kind="ExternalInput")
    ones8_d = nc.dram_tensor("ones8", [P, 64], f8, kind="ExternalInput")
    id64_d = nc.dram_tensor("id64", [64, 64], f32, kind="ExternalInput")
    out_d = nc.dram_tensor("out_t", [F, RPC], f32, kind="ExternalOutput")

    with tile.TileContext(nc) as tc, ExitStack() as ctx:
        cpool = ctx.enter_context(tc.tile_pool(name="const", bufs=1))
        wt_sb = cpool.tile([F, F], bf16, name="wt_sb")
        b4_sb = cpool.tile([1, 4 * F], f32, name="b4_sb")
        ones_r = cpool.tile([1, P], f32, name="ones_r")
        ones8 = cpool.tile([P, 2, 32], f8, name="ones8")
        id64 = cpool.tile([64, 64], f32, name="id64")
        b_rep = cpool.tile([P, 4 * F], f32, name="b_rep")
        sup_all = cpool.tile([P, JT * F], bf16, name="sup_all")
        s_all = cpool.tile([P, JT * F], bf16, name="s_all")
        d32 = cpool.tile([32, RPC], f32, name="d32")
        dinv_sb = cpool.tile([P, JT], f32, name="dinv_sb")
        dinv_rep = cpool.tile([P, RPC], f32, name="dinv_rep")
        rvec = cpool.tile([64, P], f32, name="rvec")
        out_sb = cpool.tile([P, RPC], f32, name="out_sb")

        dram = ctx.enter_context(tc.tile_pool(name="dram", bufs=1, space="DRAM"))
        ag_in = dram.tile([1, RPC], f32, name="ag_in")
        ag_out = dram.tile([NCORES, RPC], f32, name="ag_out", addr_space="Shared")

        nc.gpsimd.dma_start(wt_sb[:], wt_d[:])
        nc.gpsimd.dma_start(b4_sb[:], b4_d[:])
        nc.gpsimd.dma_start(ones_r[:], ones_r_d[:])
        nc.gpsimd.dma_start(
            ones8[:], ones8_d[:].rearrange("p (a b) -> p a b", b=32))
        nc.gpsimd.dma_start(id64[:], id64_d[:])

        a8pool = ctx.enter_context(tc.tile_pool(name="a8p", bufs=A8_BUFS))
        abpool = ctx.enter_context(tc.tile_pool(name="abp", bufs=QUADS))
        xpool = ctx.enter_context(tc.tile_pool(name="xts", bufs=XT_BUFS))
        dma_engs = [nc.sync, nc.scalar]

        ab_tiles = []

        def emit_ab(t):
            tb = abpool.tile([P, 4, RPC], bf16, name=f"ab_{t}", tag="ab")
            dma_engs[t % 2].dma_start(
                tb[:],
                ab_d[t * P:(t + 1) * P, :].rearrange(
                    "p (s i) -> p s i", s=4))
            ab_tiles.append(tb)

        with (
            tc.tile_pool(name="psum_r", bufs=1, space="PSUM") as psum_r,
            tc.tile_pool(name="psum_s", bufs=3, space="PSUM") as psum_s,
            tc.tile_pool(name="psum_b", bufs=1, space="PSUM") as psum_b,
        ):
            # ---- Phase A: degree row sums off the fp8 scout stream ----
            pr = psum_r.tile([32, RPC], f32, name="pr")
            for g in range(OCTS):
                t8 = a8pool.tile([P, 8, RPC], f8, name=f"a8_{g}", tag="a8")
                dma_engs[g % 2].dma_start(
                    t8[:],
                    a8_d[g * P:(g + 1) * P, :].rearrange(
                        "p (s i) -> p s i", s=8))
                with tc.high_priority():
                    for k in range(4):
                        for h in range(0, RPC, 512):
                            nc.tensor.matmul(
                                pr[:, h:h + 512], ones8[:],
                                t8[:, 2 * k:2 * k + 2, h:h + 512],
                                start=(g == 0 and k == 0),
                                stop=(g == OCTS - 1 and k == 3),
                                perf_mode=mybir.MatmulPerfMode.DoubleRow,
                            )

            # dinv = deg ** -0.5 on 32 duplicate rows (32 ACT/DVE lanes)
            with tc.high_priority():
                nc.scalar.sqrt(d32[:], pr[:])
                nc.vector.reciprocal_approx_fast(d32[:], d32[:])
                nc.gpsimd.dma_start(ag_in[:], d32[0:1, :])
                nc.gpsimd.collective_compute(
                    "AllGather",
                    mybir.AluOpType.bypass,
                    replica_groups=[list(range(NCORES))],
                    ins=[ag_in.opt()],
                    outs=[ag_out.opt()],
                )

            # bf16 stream, first chunk (queued behind the scout per HW ring)
            for t in range(AB_SPLIT):
                emit_ab(t)

            # bias broadcast via outer product: ones_r.T @ bias4 -> [P, 4F]
            pb = psum_b.tile([P, 4 * F], f32, name="pb")
            nc.tensor.matmul(pb[:], ones_r[:], b4_sb[:], start=True, stop=True)
            nc.vector.tensor_copy(b_rep[:], pb[:])

            # support tiles: sup[j] = x_j @ W.T + b, batched 4 tiles per psum.
            # xt rides the HW rings here, so sup lands in the AllGather window.
            for jq in range(JT // 4):
                ps = psum_s.tile([P, 4 * F], f32, name=f"ps{jq}", tag="ps")
                for k in range(4):
                    j = jq * 4 + k
                    xts = xpool.tile([F, F], bf16, name=f"xts{j}", tag="xts")
                    dma_engs[j % 2].dma_start(xts[:], xt_d[:, j * F:(j + 1) * F])
                    nc.tensor.matmul(ps[:, k * F:(k + 1) * F], xts[:], wt_sb[:],
                                     start=True, stop=True)
                sl = slice(jq * 4 * F, (jq + 1) * 4 * F)
                nc.vector.tensor_add(sup_all[:, sl], ps[:], b_rep[:])

            # bf16 stream, rest
            for t in range(AB_SPLIT, QUADS):
                emit_ab(t)

        # post-AG: load dinv as [64, 128] (sync ring, after its ab share),
        # PE-transpose to partition-major
        nc.sync.dma_start(
            rvec[:], ag_out[:].rearrange("c (a b) -> (c a) b", b=P))

        with (
            tc.tile_pool(name="psum_d", bufs=1, space="PSUM") as psum_d,
            tc.tile_pool(name="psum_t", bufs=1, space="PSUM") as psum_t,
            tc.tile_pool(name="psum_o", bufs=1, space="PSUM") as psum_o,
        ):
            pt = psum_t.tile([P, 64], f32, name="pt")
            with tc.high_priority():
                nc.tensor.transpose(pt[:], rvec[:], id64[:])
                nc.vector.tensor_copy(dinv_sb[:], pt[:])

            # scale support columns: S[j] = dinv[j] * sup[j]
            for j in range(JT):
                sl = slice(j * F, (j + 1) * F)
                nc.vector.tensor_scalar_mul(
                    s_all[:, sl], sup_all[:, sl], dinv_sb[:, j:j + 1])

            # ---- Phase D: out.T = sum_j S[j].T @ a_hat.T[j] (SBUF-resident) ----
            po = psum_o.tile([F, RPC], f32, name="po")
            for q in range(QUADS):
                for s in range(4):
                    j = 4 * q + s
                    sl = slice(j * F, (j + 1) * F)
                    for h in range(0, RPC, 512):
                        nc.tensor.matmul(
                            po[:, h:h + 512], s_all[:, sl],
                            ab_tiles[q][:, s, h:h + 512],
                            start=(j == 0), stop=(j == JT - 1),
                        )

            # local row-scale broadcast dinv_rep = ones_r.T x dinv (during AG)
            pd = psum_d.tile([F, RPC], f32, name="pd")
            for h in range(0, RPC, 512):
                nc.tensor.matmul(pd[:, h:h + 512], ones_r[:], d32[0:1, h:h + 512],
                                 start=True, stop=True)
            nc.vector.tensor_copy(dinv_rep[:], pd[:])

            # ---- Phase E: out = dinv[i] * out ----
            for h in range(0, RPC, 512):
                nc.vector.tensor_mul(out_sb[:, h:h + 512], po[:, h:h + 512],
                                     dinv_rep[:, h:h + 512])
                dma_engs[(h // 512) % 2].dma_start(
                    out_d[:, h:h + 512], out_sb[:, h:h + 512])

    nc.compile()
    return nc


_module_cache = {}


def _get_module():
    if "nc" not in _module_cache:
        nc = build_module()
        from concourse.bass_interp import get_hw_module

        nc.m = get_hw_module(nc.m)
        _module_cache["nc"] = nc
    return _module_cache["nc"]


def make_in_maps(x, adjacency, W, b):
    import ml_dtypes

    bf16 = ml_dtypes.bfloat16
    f8 = ml_dtypes.float8_e4m3

    x = np.asarray(x, dtype=np.float32)
    adjacency = np.asarray(adjacency, dtype=np.float32)
    W = np.asarray(W, dtype=np.float32)
    b = np.asarray(b, dtype=np.float32)

    xtb = np.ascontiguousarray(x.T).astype(bf16)
    wtb = np.ascontiguousarray(W.T).astype(bf16)
    bias4 = np.ascontiguousarray(np.tile(b, 4).reshape(1, 4 * F))
    ones_r = np.ones((1, P), dtype=np.float32)
    ones8 = np.ones((P, 64), dtype=f8)
    id64 = np.eye(64, dtype=np.float32)

    in_maps = []
    for c in range(NCORES):
        at = np.ascontiguousarray(adjacency[c * RPC:(c + 1) * RPC, :].T)
        # fold a_hat = A + I into the shard: global row c*RPC+i, column c*RPC+i
        at[c * RPC + np.arange(RPC), np.arange(RPC)] += 1.0
        # oct-tile-major fp8: row g*128+p, col s*1024+i <-> at[g*1024+s*128+p, i]
        a8 = at.reshape(OCTS, 8, P, RPC).transpose(0, 2, 1, 3)
        a8 = np.ascontiguousarray(a8).reshape(OCTS * P, 8 * RPC).astype(f8)
        # quad-tile-major bf16: row q*128+p, col s*1024+i <-> at[q*512+s*128+p, i]
        ab = at.reshape(QUADS, 4, P, RPC).transpose(0, 2, 1, 3)
        ab = np.ascontiguousarray(ab).reshape(QUADS * P, 4 * RPC).astype(bf16)
        in_maps.append({
            "a8": a8, "ab": ab,
            "xt": xtb, "wt": wtb, "bias4": bias4,
            "ones_r": ones_r, "ones8": ones8, "id64": id64,
        })
    return in_maps


def kernel(x, adjacency, W, b):
    from concourse.bass_utils import run_bass_kernel_spmd

    nc = _get_module()
    in_maps = make_in_maps(x, adjacency, W, b)
    res = run_bass_kernel_spmd(nc, in_maps, core_ids=list(range(NCORES)))
    out = np.empty((N, F), dtype=np.float32)
    for c in range(NCORES):
        out[c * RPC:(c + 1) * RPC, :] = res.results[c]["out_t"].T
    return out


# revision 10
# speedup vs baseline: 1.1493x; 1.1493x over previous
"""GCN layer (normalized adjacency aggregation) on 8 Trainium2 NeuronCores.

Algorithm (row-sharded, single bf16 residency + fp8 degree scout):
    a_hat = A + I  (identity folded into the shard on the host)
    deg[i] = sum_j a_hat[i, j]     -> fp8 scout copy, DoubleRow PE matmuls
    dinv = deg ** -0.5             -> sqrt + fast-reciprocal, AllGather (4KB)
    sup = x @ W.T + b              -> computed redundantly per core (tiny)
    S = dinv[:, None] * sup
    out[i, :] = dinv[i] * (a_hat[i, :] @ S)

Two copies of the row block are shipped, both transposed (contraction dim j on
partitions) and tile-major with 8KB-contiguous per-partition rows (big DMA
descriptors amortize the per-descriptor overhead):
  * a8  fp8 e4m3 in 8 oct tiles [128, 8, 1024]; 64 DoubleRow ones-matmuls
    produce the degree row sums.  fp8 degree error is ~2e-4 relative.
  * ab  bf16 in 16 quad tiles [128, 4, 1024], streamed second and kept fully
    SBUF-resident (16 MB); the aggregation pass has zero HBM re-reads.
The scout exists so the degree scan (and the AllGather, whose fixed cost is
~13us issue + ~28us execute) overlaps the bf16 stream instead of sitting
exposed between the two PE passes.

Queue plan (only three DMA queues exist: SP-HW, Act-HW, Pool-SW):
  * big streams alternate between the two HW DGE queues; a descriptor that
    waits on a semaphore head-of-line blocks its whole queue, so the
    dinv/AllGather staging lives on the Pool (gpsimd) software queue.
  * xt is emitted mid-ab-stream so the support matmuls become ready during
    the AllGather window instead of competing with the scout scan.
"""

import numpy as np
from contextlib import ExitStack

N = 8192
F = 128
NCORES = 8
RPC = N // NCORES  # 1024 rows per core
P = 128            # partitions
JT = N // P        # 64 j-tiles of 128 columns
PAIRS = JT // 2    # 32 fp8 scout tiles of 256 columns
QUADS = JT // 4    # 16 bf16 tiles of 512 columns

A8_BUFS = 10       # streaming pool for the fp8 scout tiles (2KB/part each)
AB_SPLIT = 10      # ab tiles emitted before xt (xt lands mid-stream)


def build_module():
    """Build and compile the SPMD Bass module (same program on every core)."""
    from concourse import bass, bacc, tile

    mybir = bass.mybir
    f32 = mybir.dt.float32
    bf16 = mybir.dt.bfloat16
    f8 = mybir.dt.float8e4

    nc = bacc.Bacc(
        "TRN2",
        target_bir_lowering=False,
        debug=False,
        enable_asserts=False,
        num_devices=NCORES,
    )

    a8_d = nc.dram_tensor("a8", [PAIRS * P, 2 * RPC], f8, kind="ExternalInput")
    ab_d = nc.dram_tensor("ab", [QUADS * P, 4 * RPC], bf16, kind="ExternalInput")
    xt_d = nc.dram_tensor("xt", [F, N], bf16, kind="ExternalInput")
    wt_d = nc.dram_tensor("wt", [F, F], bf16, kind="ExternalInput")
    b4_d = nc.dram_tensor("bias4", [1, 4 * F], f32, kind="ExternalInput")
    ones_r_d = nc.dram_tensor("ones_r", [1, P], f32, kind="ExternalInput")
    ones8_d = nc.dram_tensor("ones8", [P, 64], f8, kind="ExternalInput")
    id64_d = nc.dram_tensor("id64", [64, 64], f32, kind="ExternalInput")
    out_d = nc.dram_tensor("out_t", [F, RPC], f32, kind="ExternalOutput")

    with tile.TileContext(nc) as tc, ExitStack() as ctx:
        cpool = ctx.enter_context(tc.tile_pool(name="const", bufs=1))
        wt_sb = cpool.tile([F, F], bf16, name="wt_sb")
        b4_sb = cpool.tile([1, 4 * F], f32, name="b4_sb")
        ones_r = cpool.tile([1, P], f32, name="ones_r")
        ones8 = cpool.tile([P, 2, 32], f8, name="ones8")
        id64 = cpool.tile([64, 64], f32, name="id64")
        b_rep = cpool.tile([P, 4 * F], f32, name="b_rep")
        sup_all = cpool.tile([P, JT * F], bf16, name="sup_all")
        d32 = cpool.tile([32, RPC], f32, name="d32")
        dinv_sb = cpool.tile([P, JT], f32, name="dinv_sb")
        dinv_rep = cpool.tile([P, RPC], f32, name="dinv_rep")
        rvec = cpool.tile([64, P], f32, name="rvec")
        warm = cpool.tile([1, P], f32, name="warm")
        out_sb = cpool.tile([P, RPC], f32, name="out_sb")

        dram = ctx.enter_context(tc.tile_pool(name="dram", bufs=1, space="DRAM"))
        ag_in = dram.tile([1, RPC], f32, name="ag_in")
        ag_out = dram.tile([NCORES, RPC], f32, name="ag_out", addr_space="Shared")

        nc.gpsimd.dma_start(wt_sb[:], wt_d[:])
        nc.gpsimd.dma_start(b4_sb[:], b4_d[:])
        nc.gpsimd.dma_start(ones_r[:], ones_r_d[:])
        nc.gpsimd.dma_start(
            ones8[:], ones8_d[:].rearrange("p (a b) -> p a b", b=32))
        nc.gpsimd.dma_start(id64[:], id64_d[:])

        a8pool = ctx.enter_context(tc.tile_pool(name="a8p", bufs=A8_BUFS))
        abpool = ctx.enter_context(tc.tile_pool(name="abp", bufs=QUADS))
        xbig = cpool.tile([F, N], bf16, name="xbig")
        dma_engs = [nc.sync, nc.scalar]

        ab_tiles = []

        def emit_ab(t):
            tb = abpool.tile([P, 4, RPC], bf16, name=f"ab_{t}", tag="ab")
            dma_engs[t % 2].dma_start(
                tb[:],
                ab_d[t * P:(t + 1) * P, :].rearrange(
                    "p (s i) -> p s i", s=4))
            ab_tiles.append(tb)

        with (
            tc.tile_pool(name="psum_r", bufs=1, space="PSUM") as psum_r,
            tc.tile_pool(name="psum_s", bufs=3, space="PSUM") as psum_s,
            tc.tile_pool(name="psum_b", bufs=1, space="PSUM") as psum_b,
        ):
            # ---- Phase A: degree row sums off the fp8 scout stream ----
            # warm the ACT sqrt table while the scout streams
            nc.scalar.sqrt(warm[:], ones_r[:])
            pr = psum_r.tile([32, RPC], f32, name="pr")
            for t in range(PAIRS):
                t8 = a8pool.tile([P, 2, RPC], f8, name=f"a8_{t}", tag="a8")
                dma_engs[t % 2].dma_start(
                    t8[:],
                    a8_d[t * P:(t + 1) * P, :].rearrange(
                        "p (s i) -> p s i", s=2))
                with tc.high_priority():
                    for h in range(0, RPC, 512):
                        nc.tensor.matmul(
                            pr[:, h:h + 512], ones8[:], t8[:, :, h:h + 512],
                            start=(t == 0), stop=(t == PAIRS - 1),
                            perf_mode=mybir.MatmulPerfMode.DoubleRow,
                        )

            # dinv = deg ** -0.5 on 32 duplicate rows (32 ACT/DVE lanes)
            with tc.high_priority():
                nc.scalar.sqrt(d32[:], pr[:])
                nc.vector.reciprocal_approx_fast(d32[:], d32[:])
                nc.gpsimd.dma_start(ag_in[:], d32[0:1, :])
                nc.gpsimd.collective_compute(
                    "AllGather",
                    mybir.AluOpType.bypass,
                    replica_groups=[list(range(NCORES))],
                    ins=[ag_in.opt()],
                    outs=[ag_out.opt()],
                )

            # bf16 stream, first chunk (queued behind the scout per HW ring)
            for t in range(AB_SPLIT):
                emit_ab(t)

            # bias broadcast via outer product: ones_r.T @ bias4 -> [P, 4F]
            pb = psum_b.tile([P, 4 * F], f32, name="pb")
            nc.tensor.matmul(pb[:], ones_r[:], b4_sb[:], start=True, stop=True)
            nc.vector.tensor_copy(b_rep[:], pb[:])

            # support tiles: sup[j] = x_j @ W.T + b, batched 4 tiles per psum.
            # xt rides the HW rings here, so sup lands in the AllGather window.
            for q in range(4):
                dma_engs[q % 2].dma_start(
                    xbig[:, q * 2048:(q + 1) * 2048],
                    xt_d[:, q * 2048:(q + 1) * 2048])
            for jq in range(JT // 4):
                ps = psum_s.tile([P, 4 * F], f32, name=f"ps{jq}", tag="ps")
                for k in range(4):
                    j = jq * 4 + k
                    nc.tensor.matmul(ps[:, k * F:(k + 1) * F],
                                     xbig[:, j * F:(j + 1) * F], wt_sb[:],
                                     start=True, stop=True)
                sl = slice(jq * 4 * F, (jq + 1) * 4 * F)
                nc.vector.tensor_add(sup_all[:, sl], ps[:], b_rep[:])

            # bf16 stream, rest
            for t in range(AB_SPLIT, QUADS):
                emit_ab(t)

        # post-AG: load dinv as [64, 128], PE-transpose to partition-major
        nc.gpsimd.dma_start(
            rvec[:], ag_out[:].rearrange("c (a b) -> (c a) b", b=P))

        with (
            tc.tile_pool(name="psum_d", bufs=1, space="PSUM") as psum_d,
            tc.tile_pool(name="psum_t", bufs=1, space="PSUM") as psum_t,
            tc.tile_pool(name="psum_o", bufs=1, space="PSUM") as psum_o,
        ):
            pt = psum_t.tile([P, 64], f32, name="pt")
            with tc.high_priority():
                nc.tensor.transpose(pt[:], rvec[:], id64[:])
                nc.vector.tensor_copy(dinv_sb[:], pt[:])

            # scale support columns in place: S[j] = dinv[j] * sup[j]
            for j in range(JT):
                sl = slice(j * F, (j + 1) * F)
                nc.vector.tensor_scalar_mul(
                    sup_all[:, sl], sup_all[:, sl], dinv_sb[:, j:j + 1])

            # ---- Phase D: out.T = sum_j S[j].T @ a_hat.T[j] (SBUF-resident) ----
            po = psum_o.tile([F, RPC], f32, name="po")
            for q in range(QUADS):
                for s in range(4):
                    j = 4 * q + s
                    sl = slice(j * F, (j + 1) * F)
                    for h in range(0, RPC, 512):
                        nc.tensor.matmul(
                            po[:, h:h + 512], sup_all[:, sl],
                            ab_tiles[q][:, s, h:h + 512],
                            start=(j == 0), stop=(j == JT - 1),
                        )

            # local row-scale broadcast dinv_rep = ones_r.T x dinv (during AG)
            pd = psum_d.tile([F, RPC], f32, name="pd")
            for h in range(0, RPC, 512):
                nc.tensor.matmul(pd[:, h:h + 512], ones_r[:], d32[0:1, h:h + 512],
                                 start=True, stop=True)
            nc.vector.tensor_copy(dinv_rep[:], pd[:])

            # ---- Phase E: out = dinv[i] * out ----
            for h in range(0, RPC, 512):
                nc.vector.tensor_mul(out_sb[:, h:h + 512], po[:, h:h + 512],
                                     dinv_rep[:, h:h + 512])
                dma_engs[(h // 512) % 2].dma_start(
                    out_d[:, h:h + 512], out_sb[:, h:h + 512])

    nc.compile()
    return nc


_module_cache = {}


def _get_module():
    if "nc" not in _module_cache:
        nc = build_module()
        from concourse.bass_interp import get_hw_module

        nc.m = get_hw_module(nc.m)
        _module_cache["nc"] = nc
    return _module_cache["nc"]


def make_in_maps(x, adjacency, W, b):
    import ml_dtypes

    bf16 = ml_dtypes.bfloat16
    f8 = ml_dtypes.float8_e4m3

    x = np.asarray(x, dtype=np.float32)
    adjacency = np.asarray(adjacency, dtype=np.float32)
    W = np.asarray(W, dtype=np.float32)
    b = np.asarray(b, dtype=np.float32)

    xtb = np.ascontiguousarray(x.T).astype(bf16)
    wtb = np.ascontiguousarray(W.T).astype(bf16)
    bias4 = np.ascontiguousarray(np.tile(b, 4).reshape(1, 4 * F))
    ones_r = np.ones((1, P), dtype=np.float32)
    ones8 = np.ones((P, 64), dtype=f8)
    id64 = np.eye(64, dtype=np.float32)

    in_maps = []
    for c in range(NCORES):
        at = np.ascontiguousarray(adjacency[c * RPC:(c + 1) * RPC, :].T)
        # fold a_hat = A + I into the shard: global row c*RPC+i, column c*RPC+i
        at[c * RPC + np.arange(RPC), np.arange(RPC)] += 1.0
        # pair-tile-major fp8: row t*128+p, col s*1024+i <-> at[t*256+s*128+p, i]
        a8 = at.reshape(PAIRS, 2, P, RPC).transpose(0, 2, 1, 3)
        a8 = np.ascontiguousarray(a8).reshape(PAIRS * P, 2 * RPC).astype(f8)
        # quad-tile-major bf16: row q*128+p, col s*1024+i <-> at[q*512+s*128+p, i]
        ab = at.reshape(QUADS, 4, P, RPC).transpose(0, 2, 1, 3)
        ab = np.ascontiguousarray(ab).reshape(QUADS * P, 4 * RPC).astype(bf16)
        in_maps.append({
            "a8": a8, "ab": ab,
            "xt": xtb, "wt": wtb, "bias4": bias4,
            "ones_r": ones_r, "ones8": ones8, "id64": id64,
        })
    return in_maps


def kernel(x, adjacency, W, b):
    from concourse.bass_utils import run_bass_kernel_spmd

    nc = _get_module()
    in_maps = make_in_maps(x, adjacency, W, b)
    res = run_bass_kernel_spmd(nc, in_maps, core_ids=list(range(NCORES)))
    out = np.empty((N, F), dtype=np.float32)
    for c in range(NCORES):
        out[c * RPC:(c + 1) * RPC, :] = res.results[c]["out_t"].T
    return out
